# revision 1
# baseline (speedup 1.0000x reference)
"""AllAttention kernel for Trainium2 (8 NeuronCores, pure data parallel).

Computation (per batch item b):
    att   = feats[b] @ Wf + bf            # [A, H]
    att_h = h[b] @ Wh + bh                # [H]
    dot   = tanh(att + att_h)             # [A, H]
    s     = dot @ wa (+ ba)               # [A]   (ba dropped: softmax shift-invariant)
    w     = softmax(s)                    # [A]
    out   = w @ feats[b]                  # [R]

Shapes: B=256, A=196, R=1024, H=512. Sharded: batch/8 per core (32 each).

Per-core design (pairs of batch items flow through a 4-stage pipeline):
  load   : fp32 feats for 2 batches -> SBUF (SWDGE so plain loads never
           queue behind transposes)
  xform  : DVE cast fp32->bf16, then 2 batched SBUF->SBUF DMA-xbar
           transposes ([128, 2048] -> [128, 16, 128] in ONE instruction)
           to build feats^T without touching PE/ACT
  merge  : DVE-merge the a0/a1 transpose blocks into one [128, 16, 196]
           tile so mm1 needs a single matmul (single weight load) per
           (r-chunk, h-chunk)
  compute: mm1 att^T = Wf^T @ feats^T pair-packed (bf16, N=392, Wf
           stationary, FWL), tanh+bias fused on ACT (bias = per-partition
           beta^T column, beta precomputed on PE at setup), scores via PE
           with wa stationary, exp+sum fused on ACT (accum_out), softmax
           normalize on partition 0 (DVE), tiny PE transposes turn the
           weight row into columns, then mm2 out = w @ feats in bf16 with
           M=32 padded weight blocks so each 4-batch group lands on psum
           partitions {0,32,64,96} (tile_position) and drains with one
           ACT copy + strided DMA store.
The softmax/mm2 tail of pair p is interleaved between the mm1 chunks of
pair p+1 so the PE never waits on ACT's exp latency.
"""

import os
from contextlib import ExitStack

import numpy as np

import concourse.bass as bass
import concourse.bacc as bacc
import concourse.mybir as mybir
import concourse.tile as tile
from concourse.bass import ds, ts
from concourse.bass_utils import run_bass_kernel_spmd
from concourse.masks import make_identity

F32 = mybir.dt.float32
F32R = mybir.dt.float32r
BF16 = mybir.dt.bfloat16
TANH = mybir.ActivationFunctionType.Tanh
EXP = mybir.ActivationFunctionType.Exp

B, A, R, H = 256, 196, 1024, 512
N_CORES = 8
BL = B // N_CORES          # 32 batch items per core
A0 = 128                   # first a-chunk
A1 = A - A0                # 68
A1P = 80                   # padded a1 (multiple of 16 for xbar transpose)
APAD = A0 + A1P            # 224 cols per batch in featsT tiles
RC = R // 128              # 8 r-chunks
HC = H // 128              # 4 h-chunks
NPAIRS = BL // 2
NGROUPS = BL // 4


def _emit(tc):
    nc = tc.nc
    ctx = ExitStack()

    h_d = nc.dram_tensor("h_in", [BL, R], F32, kind="ExternalInput").ap()
    feats_d = nc.dram_tensor("feats_in", [BL, A, R], F32, kind="ExternalInput").ap()
    wf_d = nc.dram_tensor("wf_in", [R, H], F32, kind="ExternalInput").ap()
    bf_d = nc.dram_tensor("bf_in", [H], F32, kind="ExternalInput").ap()
    wh_d = nc.dram_tensor("wh_in", [R, H], F32, kind="ExternalInput").ap()
    bh_d = nc.dram_tensor("bh_in", [H], F32, kind="ExternalInput").ap()
    wa_d = nc.dram_tensor("wa_in", [H], F32, kind="ExternalInput").ap()
    out_d = nc.dram_tensor("out", [BL, R], F32, kind="ExternalOutput").ap()

    singles = ctx.enter_context(tc.tile_pool(name="singles", bufs=1))

    ident = singles.tile([128, 128], F32)
    make_identity(nc, ident)

    wa_sb = singles.tile([128, HC], BF16)       # wa[128*c + p] -> [p, c]
    ones_col = singles.tile([128, 1], BF16)
    nc.vector.memset(ones_col, 1.0)
    betaT = singles.tile([128, HC, BL], F32)    # beta^T[h, b] per h-chunk
    wf_bf = singles.tile([128, RC, H], BF16)    # Wf as bf16, r on partitions

    # ---- main pools ----
    fnat = ctx.enter_context(tc.tile_pool(name="fnat", bufs=3))
    fbf = ctx.enter_context(tc.tile_pool(name="fbf", bufs=4))
    ftp = ctx.enter_context(tc.tile_pool(name="ftp", bufs=2))
    dtp = ctx.enter_context(tc.tile_pool(name="dtp", bufs=2))
    erow = ctx.enter_context(tc.tile_pool(name="erow", bufs=2))
    ecol = ctx.enter_context(tc.tile_pool(name="ecol", bufs=3))
    rsb = ctx.enter_context(tc.tile_pool(name="rsb", bufs=2))
    stage = ctx.enter_context(tc.tile_pool(name="stage", bufs=2))

    mp_ps = ctx.enter_context(tc.tile_pool(name="mp_ps", bufs=2, space="PSUM"))
    sc_ps = ctx.enter_context(tc.tile_pool(name="sc_ps", bufs=2, space="PSUM"))
    res_ps = ctx.enter_context(tc.tile_pool(name="res_ps", bufs=4, space="PSUM"))

    rows4 = slice(0, 97, 32)  # partitions {0, 32, 64, 96} (DMA gather only)

    def stage_load(b0):
        fc0 = fnat.tile([128, 2, R], F32, tag="fc0")
        fc1 = fnat.tile([A1, 2, R], F32, tag="fc1")
        nc.gpsimd.dma_start(
            out=fc0, in_=feats_d[b0 : b0 + 2, 0:A0, :].rearrange("s p r -> p s r")
        )
        nc.gpsimd.dma_start(
            out=fc1, in_=feats_d[b0 : b0 + 2, A0:A, :].rearrange("s p r -> p s r")
        )
        return fc0, fc1

    def stage_xform(fc0, fc1):
        """Cast to bf16, batched xbar transposes into packed scratch.

        tp_v[:, s*RC+rc, :] = feats[b0+s][0:128, ts(rc,128)].T   (bf16)
        tq_v[:, s*RC+rc, 0:A1] = feats[b0+s][128:196, ts(rc,128)].T
        """
        fbig0 = fbf.tile([128, 2, R], BF16, tag="fb0", bufs=4)
        fbig1 = fbf.tile([A1P, 2, R], BF16, tag="fb1", bufs=4)
        nc.gpsimd.memset(fbig1[64:A1P, :, :], 0.0)
        nc.vector.tensor_copy(out=fbig0, in_=fc0)
        nc.vector.tensor_copy(out=fbig1[0:A1], in_=fc1)
        tcat = ftp.tile([128, 2 * RC * (128 + A1P)], BF16, tag="tcat", bufs=2)
        tp_v = tcat[:, 0 : 2 * RC * 128].rearrange("p (t j) -> p t j", t=2 * RC)
        tq_v = tcat[:, 2 * RC * 128 :].rearrange("p (t j) -> p t j", t=2 * RC)
        nc.sync.dma_start(
            out=tp_v, in_=fbig0.rearrange("p s r -> p (s r)"), transpose=True
        )
        nc.sync.dma_start(
            out=tq_v, in_=fbig1.rearrange("p s r -> p (s r)"), transpose=True
        )
        return fbig0, fbig1, tp_v, tq_v

    def stage_merge(xf):
        """DVE-merge the a0 transpose block into the combined [128, 16, A]
        tile; the T-b-dependent a1 part is emitted separately at iteration
        end (stage_merge2) so it never head-blocks the DVE queue."""
        fbig0, fbig1, tp_v, tq_v = xf
        Tbig = ftp.tile([128, 2 * RC, A], BF16, tag="Tbig", bufs=4)
        nc.vector.tensor_copy(out=Tbig[:, :, 0:A0], in_=tp_v)
        return fbig0, fbig1, Tbig, tq_v

    def stage_merge2(m):
        fbig0, fbig1, Tbig, tq_v = m
        nc.vector.tensor_copy(out=Tbig[:, :, A0:A], in_=tq_v[:, :, 0:A1])
        return fbig0, fbig1, Tbig

    # ---- setup: weights, h transpose, fused bias beta ----
    with tc.tile_pool(name="setup_sb", bufs=1) as setup_sb:
        # h [BL, R] -> hT [128, rc, BL] via PE transposes
        h_sb = setup_sb.tile([BL, R], F32, tag="h_sb")
        nc.sync.dma_start(out=h_sb, in_=h_d)
        hT_pt = sc_ps.tile([128, 512], F32, tag="sc")
        hT_ps = hT_pt[:, 0 : RC * BL]
        for rc in range(RC):
            nc.tensor.transpose(
                hT_ps[:, ts(rc, BL)], h_sb[:, ts(rc, 128)], ident[0:BL, 0:BL]
            )
        hT = setup_sb.tile([128, RC, BL], BF16, tag="hT")
        nc.vector.tensor_copy(out=hT, in_=hT_ps.rearrange("p (rc b) -> p rc b", rc=RC))

        # Wh -> bf16 for beta matmul
        wh_f = setup_sb.tile([128, RC, H], F32, tag="wh_f")
        nc.sync.dma_start(out=wh_f, in_=wh_d.rearrange("(rc p) h -> p rc h", p=128))
        wh_sb = setup_sb.tile([128, RC, H], BF16, tag="wh_sb")
        nc.vector.tensor_copy(out=wh_sb, in_=wh_f)

        # bf + bh  -> [1, H]
        bf_sb = setup_sb.tile([1, H], F32, tag="bf_sb")
        bh_sb = setup_sb.tile([1, H], F32, tag="bh_sb")
        bfh = setup_sb.tile([1, H], BF16, tag="bfh")
        nc.sync.dma_start(out=bf_sb, in_=bf_d[None, :])
        nc.sync.dma_start(out=bh_sb, in_=bh_d[None, :])
        nc.vector.tensor_add(out=bfh, in0=bf_sb, in1=bh_sb)

        ones_row = setup_sb.tile([1, BL], BF16, tag="ones_row")
        nc.vector.memset(ones_row, 1.0)

        # Wf -> bf16 (chunk by r)
        for rc in range(RC):
            wtmp = setup_sb.tile([128, H], F32, tag="wtmp", bufs=2)
            nc.sync.dma_start(out=wtmp, in_=wf_d[ts(rc, 128), :])
            nc.vector.tensor_copy(out=wf_bf[:, rc, :], in_=wtmp)

        # wa -> [p, c] bf16
        wa_f = setup_sb.tile([128, HC], F32, tag="wa_f")
        nc.sync.dma_start(out=wa_f, in_=wa_d.rearrange("(c p) -> p c", p=128))
        nc.vector.tensor_copy(out=wa_sb, in_=wa_f)

        # betaT[:, hc, :] = (Wh^T h^T + (bf+bh))  per h-chunk
        for hc in range(HC):
            bps_t = sc_ps.tile([128, 512], F32, tag="sc")
            bps = bps_t[:, 0:BL]
            for rc in range(RC):
                nc.tensor.matmul(
                    bps,
                    lhsT=wh_sb[:, rc, ts(hc, 128)],
                    rhs=hT[:, rc, :],
                    start=(rc == 0),
                    stop=False,
                )
            nc.tensor.matmul(
                bps,
                lhsT=bfh[0:1, ts(hc, 128)],
                rhs=ones_row,
                start=False,
                stop=True,
            )
            nc.vector.tensor_copy(out=betaT[:, hc, :], in_=bps)


    group_state = {}

    def mm1_tanh(pp, cur, tail_pieces, pump_cb=None):
        pair_b0 = 2 * pp
        fbig0, fbig1, Tbig = cur
        dt_t = dtp.tile([128, HC, 2, A], BF16, tag="dt_t")
        for hc in range(HC):
            if hc < len(tail_pieces):
                tail_pieces[hc]()
            if hc == 1 and pump_cb is not None:
                pump_cb()
            mp = mp_ps.tile([128, 2, A], F32, tag="mp")
            for rc in range(RC):
                nc.tensor.matmul(
                    mp,
                    lhsT=wf_bf[:, rc, ts(hc, 128)],
                    rhs=Tbig[:, rc : rc + RC + 1 : RC, :],
                    start=(rc == 0),
                    stop=(rc == RC - 1),
                )
            for s in range(2):
                nc.scalar.activation(
                    out=dt_t[:, hc, s, :],
                    in_=mp[:, s, :],
                    func=TANH,
                    bias=betaT[:, hc, pair_b0 + s : pair_b0 + s + 1],
                    scale=1.0,
                )
        for piece in tail_pieces[HC:]:
            piece()
        return fbig0, fbig1, dt_t

    def tail_pieces(pp, fbig0, fbig1, dt_t):
        """Return closures for pair pp's softmax/mm2 tail, to be interleaved
        between the next pair's mm1 chunks."""
        pair_b0 = 2 * pp
        g = pp // 2
        if pp % 2 == 0:
            res_lo = res_ps.tile([128, 512], F32, tag="res")
            res_hi = res_ps.tile([128, 512], F32, tag="res")
            group_state[g] = (res_lo, res_hi)
        res_lo, res_hi = group_state[g]
        scb = sc_ps.tile([128, 512], F32, tag="sc")
        ecs = {}

        def p_scores():
            sc = scb[0:1, 0 : 2 * A]
            for hc in range(HC):
                nc.tensor.matmul(
                    sc,
                    lhsT=wa_sb[:, hc : hc + 1],
                    rhs=dt_t[:, hc, :, :],
                    start=(hc == 0),
                    stop=(hc == HC - 1),
                )
            er = erow.tile([1, 2 * A], F32, tag="er")
            sS = rsb.tile([1, 4], F32, tag="sS")
            for s in range(2):
                nc.scalar.activation(
                    out=er[0:1, ts(s, A)], in_=sc[0:1, ds(s * A, A)], func=EXP,
                    accum_out=sS[0:1, s : s + 1],
                )
            ecs["er"] = er
            ecs["sS"] = sS

        def p_norm():
            # softmax normalize on partition 0: w = exp / sum(exp)
            er = ecs["er"]
            sS = ecs["sS"]
            nc.vector.reciprocal(out=sS[0:1, 2:4], in_=sS[0:1, 0:2])
            ern = erow.tile([1, 2 * A], F32, tag="ern")
            for s in range(2):
                nc.vector.tensor_scalar_mul(
                    ern[0:1, ts(s, A)], er[0:1, ts(s, A)], sS[0:1, 2 + s : 3 + s]
                )
            ecs["ern"] = ern

        def p_expt():
            er = ecs["ern"]
            for s in range(2):
                et = scb[:, 400 + 2 * s : 402 + 2 * s]
                nc.tensor.transpose(
                    et[:, 0:1], er[0:1, ds(s * A, A0)], ident[0:1, 0:1]
                )
                nc.tensor.transpose(
                    et[0:A1, 1:2], er[0:1, ds(s * A + A0, A1)], ident[0:1, 0:1]
                )
                ec = ecol.tile([128, 2, 32], BF16, tag="ec")
                nc.gpsimd.memset(ec, 1.0)
                nc.scalar.copy(out=ec[:, 0, 0:1], in_=et[:, 0:1])
                nc.scalar.copy(out=ec[0:A1, 1, 0:1], in_=et[0:A1, 1:2])
                ecs[s] = ec

        def p_mm2(s):
            b = pair_b0 + s
            jb = b % 4
            ec = ecs[s]
            lo = ds(0, 512)
            hi = ds(512, 512)
            for res_t, cols in ((res_lo, lo), (res_hi, hi)):
                nc.tensor.matmul(
                    res_t[ds(32 * jb, 32), :],
                    lhsT=ec[:, 0, :],
                    rhs=fbig0[:, s, cols],
                    start=True,
                    stop=False,
                    tile_position=(0, 32 * jb),
                )
                nc.tensor.matmul(
                    res_t[ds(32 * jb, 32), :],
                    lhsT=ec[0:A1, 1, :],
                    rhs=fbig1[0:A1, s, cols],
                    start=False,
                    stop=True,
                    tile_position=(0, 32 * jb),
                )
        def p_drain():
            if pp % 2 != 1:
                return
            for res_t, half in ((res_lo, 0), (res_hi, 1)):
                st = stage.tile([128, 512], F32, tag="st")
                nc.scalar.copy(out=st, in_=res_t)
                nc.sync.dma_start(
                    out=out_d[ts(g, 4), ts(half, 512)], in_=st[rows4, :]
                )
            del group_state[g]

        def p_last():
            p_mm2(1)
            p_drain()

        return [p_scores, p_norm, p_expt, lambda: p_mm2(0), p_last]



    # early feats prefetch: drive pair 0 through the whole chain first so
    # the first mm1 can start ~12us in; later pairs queue behind it.
    fcs, xfs, merged = {}, {}, {}
    fcs[0] = stage_load(0)
    xfs[0] = stage_xform(*fcs.pop(0))
    fcs[1] = stage_load(2)
    merged[0] = stage_merge2(stage_merge(xfs.pop(0)))
    fcs[2] = stage_load(4)

    # ---- pipelined driver ----

    def pump(pp):
        q = pp + 2
        if 0 <= q < NPAIRS and q in xfs:
            merged[q] = stage_merge(xfs.pop(q))
        q = pp + 4
        if 0 <= q < NPAIRS:
            fcs[q] = stage_load(2 * q)
        q = pp + 3
        if 0 <= q < NPAIRS and q in fcs:
            xfs[q] = stage_xform(*fcs.pop(q))

    def pump_tail(pp):
        q = pp + 2
        if 0 <= q < NPAIRS and q in merged and len(merged[q]) == 4:
            merged[q] = stage_merge2(merged[q])

    xfs[1] = stage_xform(*fcs.pop(1))
    fcs[3] = stage_load(6)
    merged[1] = stage_merge2(stage_merge(xfs.pop(1)))
    xfs[2] = stage_xform(*fcs.pop(2))

    SAME_PAIR_TAIL = False
    if SAME_PAIR_TAIL:
        for pp in range(NPAIRS):
            cur = merged.pop(pp)
            out = mm1_tanh(pp, cur, [], pump_cb=lambda pp=pp: pump(pp))
            for piece in tail_pieces(pp, *out):
                piece()
    else:
        prev = None
        for pp in range(NPAIRS):
            cur = merged.pop(pp)
            assert len(cur) == 3, f"pair {pp} missing merge2"
            pieces = tail_pieces(*prev) if prev is not None else []
            prev_out = mm1_tanh(pp, cur, pieces, pump_cb=lambda pp=pp: pump(pp))
            pump_tail(pp)
            prev = (pp, *prev_out)
        for piece in tail_pieces(*prev):
            piece()
    ctx.close()


_CACHE = {}


def _build():
    if "nc" in _CACHE:
        return _CACHE["nc"]
    nc = bacc.Bacc(
        "TRN2",
        target_bir_lowering=False,
        debug=False,
        enable_asserts=False,
        num_devices=N_CORES,
    )
    with tile.TileContext(nc) as tc:
        _emit(tc)
    nc.compile()
    _CACHE["nc"] = nc
    return nc


def kernel(h, feats, Wf, bf, Wh, bh, wa, ba=None, **_unused):
    h = np.ascontiguousarray(np.asarray(h, dtype=np.float32))
    feats = np.ascontiguousarray(np.asarray(feats, dtype=np.float32))
    Wf = np.ascontiguousarray(np.asarray(Wf, dtype=np.float32))
    bf = np.ascontiguousarray(np.asarray(bf, dtype=np.float32))
    Wh = np.ascontiguousarray(np.asarray(Wh, dtype=np.float32))
    bh = np.ascontiguousarray(np.asarray(bh, dtype=np.float32))
    wa = np.ascontiguousarray(np.asarray(wa, dtype=np.float32))

    nc = _build()
    in_maps = []
    for i in range(N_CORES):
        sl = slice(i * BL, (i + 1) * BL)
        in_maps.append(
            {
                "h_in": np.ascontiguousarray(h[sl]),
                "feats_in": np.ascontiguousarray(feats[sl]),
                "wf_in": Wf,
                "bf_in": bf,
                "wh_in": Wh,
                "bh_in": bh,
                "wa_in": wa,
            }
        )
    res = run_bass_kernel_spmd(nc, in_maps, core_ids=list(range(N_CORES)))
    out = np.concatenate([res.results[i]["out"] for i in range(N_CORES)], axis=0)
    return out.astype(np.float32)


if __name__ == "__main__":
    rng = np.random.default_rng(0)
    s_f = 1.0 / np.sqrt(R)
    s_a = 1.0 / np.sqrt(H)
    inputs = {
        "h": rng.standard_normal((B, R), dtype=np.float32),
        "feats": rng.standard_normal((B, A, R), dtype=np.float32),
        "Wf": rng.uniform(-s_f, s_f, (R, H)).astype(np.float32),
        "bf": rng.uniform(-s_f, s_f, (H,)).astype(np.float32),
        "Wh": rng.uniform(-s_f, s_f, (R, H)).astype(np.float32),
        "bh": rng.uniform(-s_f, s_f, (H,)).astype(np.float32),
        "wa": rng.uniform(-s_a, s_a, (H,)).astype(np.float32),
        "ba": np.float32(0.1),
    }
    out = kernel(**inputs)
    print(out.shape, out.dtype, np.abs(out).mean())



# revision 70
# speedup vs baseline: 1.4414x; 1.4414x over previous
"""AllAttention kernel for Trainium2 (8 NeuronCores, pure data parallel).

Computation (per batch item b):
    att   = feats[b] @ Wf + bf            # [A, H]
    att_h = h[b] @ Wh + bh                # [H]
    dot   = tanh(att + att_h)             # [A, H]
    s     = dot @ wa (+ ba)               # [A]   (ba dropped: softmax shift-invariant)
    w     = softmax(s)                    # [A]
    out   = w @ feats[b]                  # [R]

Shapes: B=256, A=196, R=1024, H=512. Sharded: batch/8 per core (32 each).

Per-core design (pairs of batch items flow through a software pipeline):
  load   : fp32 feats for 2 batches -> SBUF via SWDGE (Pool ring)
  xform  : DVE cast fp32->bf16; a0 rows (128) transposed by one batched
           SBUF->SBUF DMA-xbar transpose (SP ring); a1 rows (68) transposed
           on the PE (16 small transpose matmuls -> PSUM -> DVE copy), so
           no pad memsets and no second xbar transpose
  mm1    : att^T = Wf^T @ feats^T, reading the xbar tile (a0) and the
           PE-transposed tile (a1) directly as two accumulation regions
           per (h-chunk); tanh+bias fused on ACT (bias = per-partition
           beta^T column, beta precomputed on PE from Wh/h/bf/bh)
  scores : PE matmul with wa stationary; exp+sum fused on ACT (accum_out);
           softmax normalize on partition 0 (DVE); tiny PE transposes turn
           the weight row into columns
  mm2    : out^T columns via N=1 matmuls with feats-natural as the
           stationary operand (Ldweights is cheap; avoids the N=512 M=32
           waste of a row-major mm2).  Results accumulate as out^T[r, b]
           in PSUM; each group of 4 batches is drained by one ACT copy,
           transposed back to [b, r] by one PE transpose, and stored with
           a single contiguous DMA.
The softmax/mm2 tail of pair p is spread over the mm1 chunks of pairs
p+1 and p+2 so the in-order PE queue never head-blocks on the
cross-engine softmax chain.  Weight loads go over the ACT DMA ring,
feats loads over the Pool ring, and xbar transposes + stores over the SP
ring so no ring head-blocks another.
"""

import os
from contextlib import ExitStack

import numpy as np

import concourse.bass as bass
import concourse.bacc as bacc
import concourse.mybir as mybir
import concourse.tile as tile
from concourse.bass import ds, ts
from concourse.bass_utils import run_bass_kernel_spmd
from concourse.masks import make_identity

F32 = mybir.dt.float32
BF16 = mybir.dt.bfloat16
TANH = mybir.ActivationFunctionType.Tanh
EXP = mybir.ActivationFunctionType.Exp

B, A, R, H = 256, 196, 1024, 512
N_CORES = 8
BL = B // N_CORES          # 32 batch items per core
A0 = 128                   # first a-chunk (DMA-xbar transposed)
A1 = A - A0                # 68  (PE transposed)
RC = R // 128              # 8 r-chunks
HC = H // 128              # 4 h-chunks
NPAIRS = BL // 2
NGROUPS = BL // 4


def _emit(tc):
    nc = tc.nc
    ctx = ExitStack()

    h_d = nc.dram_tensor("h_in", [BL, R], F32, kind="ExternalInput").ap()
    feats_d = nc.dram_tensor("feats_in", [BL, A, R], F32, kind="ExternalInput").ap()
    wf_d = nc.dram_tensor("wf_in", [R, H], F32, kind="ExternalInput").ap()
    bf_d = nc.dram_tensor("bf_in", [H], F32, kind="ExternalInput").ap()
    wh_d = nc.dram_tensor("wh_in", [R, H], F32, kind="ExternalInput").ap()
    bh_d = nc.dram_tensor("bh_in", [H], F32, kind="ExternalInput").ap()
    wa_d = nc.dram_tensor("wa_in", [H], F32, kind="ExternalInput").ap()
    out_d = nc.dram_tensor("out", [BL, R], F32, kind="ExternalOutput").ap()

    singles = ctx.enter_context(tc.tile_pool(name="singles", bufs=1))

    ident = singles.tile([128, 128], F32)
    make_identity(nc, ident)
    ident_bf = singles.tile([128, 128], BF16)
    nc.vector.tensor_copy(out=ident_bf, in_=ident)

    wa_sb = singles.tile([128, HC], BF16)       # wa[128*c + p] -> [p, c]
    betaT = singles.tile([128, HC, BL], F32)    # beta^T[h, b] per h-chunk
    wf_bf = singles.tile([128, RC, H], BF16)    # Wf bf16, r on partitions
    # out^T staging: outT_sb[:, g, jb, rc] = out[4g+jb][rc*128 + p]
    outT_sb = singles.tile([128, NGROUPS, 4, RC], F32)
    ones11 = singles.tile([1, 1], F32)
    nc.vector.memset(ones11, 1.0)
    # REP[j, j*8+i] = 1: replicates a [4,1] per-batch column to [32,1]
    rep4 = singles.tile([4, 32], F32)
    nc.gpsimd.memset(rep4, 1.0)
    nc.gpsimd.affine_select(
        out=rep4, in_=rep4, compare_op=mybir.AluOpType.is_ge, fill=0.0,
        base=0, pattern=[[1, 32]], channel_multiplier=-8,
    )
    nc.gpsimd.affine_select(
        out=rep4, in_=rep4, compare_op=mybir.AluOpType.is_ge, fill=0.0,
        base=7, pattern=[[-1, 32]], channel_multiplier=8,
    )

    # ---- main pools ----
    fnat = ctx.enter_context(tc.tile_pool(name="fnat", bufs=3))
    fbf = ctx.enter_context(tc.tile_pool(name="fbf", bufs=5))
    ftp = ctx.enter_context(tc.tile_pool(name="ftp", bufs=3))
    ta1p = ctx.enter_context(tc.tile_pool(name="ta1p", bufs=2))
    dtp = ctx.enter_context(tc.tile_pool(name="dtp", bufs=2))
    erow = ctx.enter_context(tc.tile_pool(name="erow", bufs=2))
    ecol = ctx.enter_context(tc.tile_pool(name="ecol", bufs=6))
    rsb = ctx.enter_context(tc.tile_pool(name="rsb", bufs=2))
    stage = ctx.enter_context(tc.tile_pool(name="stage", bufs=2))
    setup_sb = ctx.enter_context(tc.tile_pool(name="setup_sb", bufs=1))

    mp_ps = ctx.enter_context(tc.tile_pool(name="mp_ps", bufs=2, space="PSUM"))
    sc_ps = ctx.enter_context(tc.tile_pool(name="sc_ps", bufs=2, space="PSUM"))
    oT_ps = ctx.enter_context(tc.tile_pool(name="oT_ps", bufs=2, space="PSUM"))
    tq_ps = ctx.enter_context(tc.tile_pool(name="tq_ps", bufs=2, space="PSUM"))

    # ---- pipeline state ----
    fblks = {}   # block k -> (fc0, fc1) fp32 natural, pairs 2k and 2k+1
    fbigs = {}   # pair -> (fbig0, fbig1) bf16 natural (kept until mm2)
    tps = {}     # pair -> tp (a0 transposed, [128, 2*RC, 128])
    ta1s = {}    # pair -> ta1 (a1 transposed, [128, 2, RC, A1])
    dts = {}     # pair -> dt_t tanh output
    ecs_all = {} # pair -> {"er":..., s: ec}
    scbs = {}    # pair -> scores psum tile
    group_oT = {}

    def stage_load(k, split=False):
        # one 4-batch block per DMA pair: few SWDGE dispatches, slow
        # rotation of the 8 SW DMA-completion semaphores.  split=True uses
        # pair-sized halves so the first casts can start sooner (startup).
        b0 = 4 * k
        fc0 = fnat.tile([128, 4, R], F32, tag="fc0", name="fc0")
        fc1 = fnat.tile([A1, 4, R], F32, tag="fc1", name="fc1")
        halves = ((0, 2), (2, 4)) if split else ((0, 4),)
        for lo, hi in halves:
            # a1 rows first: the PE a1-transposes are the earliest consumer
            nc.gpsimd.dma_start(
                out=fc1[:, lo:hi, :],
                in_=feats_d[b0 + lo : b0 + hi, A0:A, :].rearrange("s p r -> p s r"),
            )
            nc.gpsimd.dma_start(
                out=fc0[:, lo:hi, :],
                in_=feats_d[b0 + lo : b0 + hi, 0:A0, :].rearrange("s p r -> p s r"),
            )
        fblks[k] = (fc0, fc1)

    def stage_xform(q):
        """Cast fp32->bf16 (DVE) and launch the a0 xbar transpose (SP ring).

        tp[:, s*RC+rc, :] = feats[2q+s][0:128, ts(rc,128)].T  (bf16)
        """
        fc0, fc1 = fblks[q // 2]
        j = 2 * (q % 2)
        fbig0 = fbf.tile([128, 2, R], BF16, tag="fb0", name="fb0")
        fbig1 = fbf.tile([A1, 2, R], BF16, tag="fb1", name="fb1")
        nc.vector.tensor_copy(out=fbig0, in_=fc0[:, j : j + 2, :])
        nc.vector.tensor_copy(out=fbig1, in_=fc1[:, j : j + 2, :])
        if q % 2 == 1:
            del fblks[q // 2]
        tp = ftp.tile([128, 2 * RC, A0], BF16, tag="tp", name="tp")
        nc.sync.dma_start(
            out=tp, in_=fbig0.rearrange("p s r -> p (s r)"), transpose=True
        )
        fbigs[q] = (fbig0, fbig1)
        tps[q] = tp

    def stage_a1t(q):
        """PE-transpose the 68-row a1 blocks into PSUM, copy to SBUF."""
        fbig1 = fbigs[q][1]
        ta1 = ta1p.tile([128, 2, RC, A1], BF16, tag="ta1", name="ta1")
        for s in range(2):
            tq = tq_ps.tile([128, RC, 72], BF16, tag="tq", name="tq")
            for rc in range(RC):
                nc.tensor.transpose(
                    tq[:, rc, 0:A1],
                    fbig1[:, s, ts(rc, 128)],
                    ident_bf[0:A1, 0:A1],
                )
            # ACT does the PSUM->SBUF merge so the DVE queue stays a pure
            # load->cast chain (no coupling to the PE/softmax clock)
            nc.scalar.copy(out=ta1[:, s], in_=tq[:, :, 0:A1])
        ta1s[q] = ta1

    # ---- setup: h / biases / wa; weight loads per h-column-block ----
    # h shares the weight-staging rotation (first allocation -> first buf)
    h_sb = setup_sb.tile([BL, R], F32, tag="ws", bufs=3, name="h_sb")
    nc.sync.dma_start(out=h_sb, in_=h_d)
    bf_sb = setup_sb.tile([1, H], F32, name="bf_sb")
    bh_sb = setup_sb.tile([1, H], F32, name="bh_sb")
    nc.sync.dma_start(out=bf_sb, in_=bf_d[None, :])
    nc.sync.dma_start(out=bh_sb, in_=bh_d[None, :])
    wa_f = setup_sb.tile([128, HC], F32, name="wa_f")
    nc.sync.dma_start(out=wa_f, in_=wa_d.rearrange("(c p) -> p c", p=128))

    # feats loads first on the Pool ring so they lead DMA arbitration;
    # xbar transposes claim their HWDGE semaphores before the weight loads
    # (otherwise tp0 serializes behind the weight/beta chain)
    stage_load(0, split=True)
    stage_xform(0)
    stage_load(1)
    stage_xform(1)
    stage_load(2)

    # weight column-blocks on the ACT ring: wf[hc] before wh[hc]; per-hc
    # availability lets mm1/tanh start before the full weights arrive.
    wcast = {}
    for hc in range(HC):
        for wd, nm in ((wf_d, "wf"), (wh_d, "wh")):
            wtmp = setup_sb.tile(
                [128, RC, 128], F32, tag="ws", bufs=3, name="wtmp"
            )
            nc.scalar.dma_start(
                out=wtmp, in_=wd[:, ts(hc, 128)].rearrange("(rc p) h -> p rc h", p=128)
            )
            wcast[(nm, hc)] = wtmp

    def cast_weights(hc):
        nc.vector.tensor_copy(
            out=wf_bf[:, :, ts(hc, 128)], in_=wcast.pop(("wf", hc))
        )

    nc.vector.tensor_copy(out=wa_sb, in_=wa_f)
    bfh = setup_sb.tile([1, H], BF16, name="bfh")
    nc.vector.tensor_add(out=bfh, in0=bf_sb, in1=bh_sb)
    ones_row = setup_sb.tile([1, BL], BF16, name="ones_row")
    nc.vector.memset(ones_row, 1.0)

    # h [BL, R] -> hT [128, rc, BL] via PE transposes (first PE work)
    hT_pt = sc_ps.tile([128, 512], F32, tag="sc", name="hT_pt")
    hT_ps = hT_pt[:, 0 : RC * BL]
    for rc in range(RC):
        nc.tensor.transpose(
            hT_ps[:, ts(rc, BL)], h_sb[:, ts(rc, 128)], ident[0:BL, 0:BL]
        )
    hT = setup_sb.tile([128, RC, BL], BF16, name="hT")
    nc.vector.tensor_copy(out=hT, in_=hT_ps.rearrange("p (rc b) -> p rc b", rc=RC))

    def beta_mm(hc):
        # betaT[:, hc, :] = Wh[:, hc-chunk]^T @ h^T + (bf+bh)[hc-chunk]
        # (Wh used as f32r straight from the fp32 staging tile)
        cast_weights(hc)
        whb = setup_sb.tile([128, RC, 128], BF16, tag="whb", bufs=2, name="whb")
        nc.vector.tensor_copy(out=whb, in_=wcast.pop(("wh", hc)))
        bps_t = sc_ps.tile([128, 512], F32, tag="sc", name="bps_t")
        bps = bps_t[:, 0:BL]
        for rc in range(RC):
            nc.tensor.matmul(
                bps,
                lhsT=whb[:, rc, :],
                rhs=hT[:, rc, :],
                start=(rc == 0),
                stop=False,
            )
        nc.tensor.matmul(
            bps,
            lhsT=bfh[0:1, ts(hc, 128)],
            rhs=ones_row,
            start=False,
            stop=True,
        )
        nc.vector.tensor_copy(out=betaT[:, hc, :], in_=bps)

    # ---- per-pair pieces ----

    def mm1_tanh(pp, slots):
        """Pair pp's mm1+tanh; slots[hc] closures run before each h-chunk."""
        tp = tps.pop(pp)
        ta1 = ta1s.pop(pp)
        dt_t = dtp.tile([128, HC, 2, A], BF16, tag="dt_t", name="dt_t")
        for hc in range(HC):
            for piece in slots[hc]:
                piece()
            # mp is a-major [128, A, 2] so the a0/a1 regions are contiguous
            mp = mp_ps.tile([128, A, 2], F32, tag="mp", name="mp")
            for rc in range(RC):
                nc.tensor.matmul(
                    mp[:, 0:A0, :],
                    lhsT=wf_bf[:, rc, ts(hc, 128)],
                    rhs=tp[:, rc : rc + RC + 1 : RC, :].rearrange("p s a -> p a s"),
                    start=(rc == 0),
                    stop=(rc == RC - 1),
                )
            for rc in range(RC):
                nc.tensor.matmul(
                    mp[:, A0:A, :],
                    lhsT=wf_bf[:, rc, ts(hc, 128)],
                    rhs=ta1[:, :, rc, :].rearrange("p s a -> p a s"),
                    start=(rc == 0),
                    stop=(rc == RC - 1),
                )
            for s in range(2):
                nc.scalar.activation(
                    out=dt_t[:, hc, s, :],
                    in_=mp[:, :, s],
                    func=TANH,
                    bias=betaT[:, hc, 2 * pp + s : 2 * pp + s + 1],
                    scale=1.0,
                )
        dts[pp] = dt_t

    grows = {}   # group -> [1, 4] row of exp-sums (one per batch)

    def p_scores(pp):
        dt_t = dts.pop(pp)
        g = pp // 2
        if pp % 2 == 0:
            grows[g] = rsb.tile([1, 4], F32, tag="grow", name="grow")
        grow = grows[g]
        scb = sc_ps.tile([128, 512], F32, tag="sc", name="scb")
        sc = scb[0:1, 0 : 2 * A]
        for hc in range(HC):
            nc.tensor.matmul(
                sc,
                lhsT=wa_sb[:, hc : hc + 1],
                rhs=dt_t[:, hc, :, :],
                start=(hc == 0),
                stop=(hc == HC - 1),
            )
        er = erow.tile([1, 2 * A], F32, tag="er", name="er")
        j0 = 2 * (pp % 2)
        for s in range(2):
            nc.scalar.activation(
                out=er[0:1, ts(s, A)], in_=sc[0:1, ds(s * A, A)], func=EXP,
                accum_out=grow[0:1, j0 + s : j0 + s + 1],
            )
        scbs[pp] = scb
        ecs_all[pp] = {"er": er}

    def p_expt(pp):
        # UNNORMALIZED exp columns via K=1 PE transpose-matmuls; the 1/sum
        # scaling happens per group in p_final where batch sits on partitions
        ecs = ecs_all[pp]
        er = ecs["er"]
        scb = scbs.pop(pp)
        for s in range(2):
            et = scb[:, 400 + 2 * s : 402 + 2 * s]
            nc.tensor.matmul(
                et[:, 0:1],
                lhsT=er[0:1, ds(s * A, A0)],
                rhs=ones11,
                start=True,
                stop=True,
            )
            nc.tensor.matmul(
                et[0:A1, 1:2],
                lhsT=er[0:1, ds(s * A + A0, A1)],
                rhs=ones11,
                start=True,
                stop=True,
            )
            ec = ecol.tile([128, 2], BF16, tag="ec", name="ec")
            nc.scalar.copy(out=ec[:, 0:1], in_=et[:, 0:1])
            nc.scalar.copy(out=ec[0:A1, 1:2], in_=et[0:A1, 1:2])
            ecs[s] = ec

    def p_mm2(pp):
        # out^T[:, jb, rc] += feats[b]^T @ w[b]; feats natural is the
        # stationary operand, the softmax column the N=1 moving rhs.
        g = pp // 2
        if pp % 2 == 0:
            group_oT[g] = oT_ps.tile([128, 4, RC], F32, tag="oT", name="oT")
        oT = group_oT[g]
        fbig0, fbig1 = fbigs.pop(pp)
        ecs = ecs_all.pop(pp)
        for s in range(2):
            jb = (2 * pp + s) % 4
            ec = ecs[s]
            for rc in range(RC):
                nc.tensor.matmul(
                    oT[:, jb, rc : rc + 1],
                    lhsT=fbig0[:, s, ts(rc, 128)],
                    rhs=ec[:, 0:1],
                    start=True,
                    stop=False,
                )
                nc.tensor.matmul(
                    oT[:, jb, rc : rc + 1],
                    lhsT=fbig1[:, s, ts(rc, 128)],
                    rhs=ec[0:A1, 1:2],
                    start=False,
                    stop=True,
                )

    def p_drain(pp):
        g = pp // 2
        nc.scalar.copy(out=outT_sb[:, g], in_=group_oT.pop(g))

    sts = {}

    def p_final(g):
        # transpose out^T group back to [b, r] rows (batch on partitions)
        # and apply the per-batch 1/sum softmax normalization
        grow = grows.pop(g)
        growr = rsb.tile([1, 4], F32, tag="growr", name="growr")
        nc.vector.reciprocal(out=growr, in_=grow)
        T = sc_ps.tile([32, 512], F32, tag="sc", name="T")
        nc.tensor.matmul(
            T[0:4, 300:301], lhsT=growr, rhs=ones11, start=True, stop=True
        )
        c4sb = stage.tile([4, 1], F32, tag="c4", name="c4sb")
        nc.scalar.copy(out=c4sb, in_=T[0:4, 300:301])
        nc.tensor.matmul(
            T[0:32, 302:303], lhsT=rep4, rhs=c4sb, start=True, stop=True
        )
        c32sb = stage.tile([32, 1], F32, tag="c32", name="c32sb")
        nc.scalar.copy(out=c32sb, in_=T[0:32, 302:303])
        nc.tensor.transpose(
            T[:, 0:128], outT_sb[:, g].rearrange("p a b -> p (a b)"), ident
        )
        st = stage.tile([32, 128], F32, tag="st", name="st")
        nc.vector.tensor_scalar_mul(st, T[:, 0:128], c32sb)
        sts[g] = st

    def p_store(g):
        # issued a pair after p_final so it never head-blocks the SP ring
        nc.sync.dma_start(
            out=out_d[ts(g, 4), :].rearrange("b (rc r) -> (b rc) r", r=128),
            in_=sts.pop(g),
        )

    # ---- prologue: fill the pipeline ----
    stage_a1t(0)

    # ---- main loop ----
    for pp in range(NPAIRS):
        slots = [[], [], [], []]
        if pp == 0:
            for hc in range(HC):
                slots[hc].append(lambda hc=hc: beta_mm(hc))
        if pp % 2 == 0 and (pp + 6) // 2 < NPAIRS // 2:
            slots[0].append(lambda k=(pp + 6) // 2: stage_load(k))
        if pp >= 2:
            slots[0].append(lambda q=pp - 2: p_mm2(q))
            if (pp - 2) % 2 == 1:
                slots[1].append(lambda q=pp - 2: p_drain(q))
        if pp + 1 < NPAIRS:
            slots[1].append(lambda q=pp + 1: stage_a1t(q))
        if pp >= 1:
            slots[1].append(lambda q=pp - 1: p_scores(q))
        if pp + 2 < NPAIRS:
            slots[2].append(lambda q=pp + 2: stage_xform(q))
        if pp >= 1:
            slots[3].append(lambda q=pp - 1: p_expt(q))
        if pp >= 4 and pp % 2 == 0:
            slots[3].append(lambda g=(pp - 4) // 2: p_final(g))
        if pp >= 5 and pp % 2 == 1:
            slots[1].append(lambda g=(pp - 5) // 2: p_store(g))
        mm1_tanh(pp, slots)

    # ---- epilogue ----
    p_scores(NPAIRS - 1)
    p_expt(NPAIRS - 1)
    p_mm2(NPAIRS - 2)
    p_mm2(NPAIRS - 1)
    p_drain(NPAIRS - 1)
    p_final(NGROUPS - 2)
    p_store(NGROUPS - 2)
    p_final(NGROUPS - 1)
    p_store(NGROUPS - 1)
    ctx.close()


_CACHE = {}


def _build():
    if "nc" in _CACHE:
        return _CACHE["nc"]
    nc = bacc.Bacc(
        "TRN2",
        target_bir_lowering=False,
        debug=False,
        enable_asserts=False,
        num_devices=N_CORES,
        dynamic_dma_scratch_size=24576,
        
    )
    with tile.TileContext(nc) as tc:
        _emit(tc)
    nc.compile()
    _CACHE["nc"] = nc
    return nc


def kernel(h, feats, Wf, bf, Wh, bh, wa, ba=None, **_unused):
    h = np.ascontiguousarray(np.asarray(h, dtype=np.float32))
    feats = np.ascontiguousarray(np.asarray(feats, dtype=np.float32))
    Wf = np.ascontiguousarray(np.asarray(Wf, dtype=np.float32))
    bf = np.ascontiguousarray(np.asarray(bf, dtype=np.float32))
    Wh = np.ascontiguousarray(np.asarray(Wh, dtype=np.float32))
    bh = np.ascontiguousarray(np.asarray(bh, dtype=np.float32))
    wa = np.ascontiguousarray(np.asarray(wa, dtype=np.float32))

    nc = _build()
    in_maps = []
    for i in range(N_CORES):
        sl = slice(i * BL, (i + 1) * BL)
        in_maps.append(
            {
                "h_in": np.ascontiguousarray(h[sl]),
                "feats_in": np.ascontiguousarray(feats[sl]),
                "wf_in": Wf,
                "bf_in": bf,
                "wh_in": Wh,
                "bh_in": bh,
                "wa_in": wa,
            }
        )
    res = run_bass_kernel_spmd(nc, in_maps, core_ids=list(range(N_CORES)))
    out = np.concatenate([res.results[i]["out"] for i in range(N_CORES)], axis=0)
    return out.astype(np.float32)


if __name__ == "__main__":
    rng = np.random.default_rng(0)
    s_f = 1.0 / np.sqrt(R)
    s_a = 1.0 / np.sqrt(H)
    inputs = {
        "h": rng.standard_normal((B, R), dtype=np.float32),
        "feats": rng.standard_normal((B, A, R), dtype=np.float32),
        "Wf": rng.uniform(-s_f, s_f, (R, H)).astype(np.float32),
        "bf": rng.uniform(-s_f, s_f, (H,)).astype(np.float32),
        "Wh": rng.uniform(-s_f, s_f, (R, H)).astype(np.float32),
        "bh": rng.uniform(-s_f, s_f, (H,)).astype(np.float32),
        "wa": rng.uniform(-s_a, s_a, (H,)).astype(np.float32),
        "ba": np.float32(0.1),
    }
    out = kernel(**inputs)
    print(out.shape, out.dtype, np.abs(out).mean())
